# revision 2
# baseline (speedup 1.0000x reference)
"""Single-head attention (B=4, S=2048, D=1024, KQ=64) on 8 trn2 NeuronCores.

Sharding: (batch, query-half) -> 8 shards. Each core computes K/V for the
full sequence of its batch and attention output for its 1024 query rows.

Per-core program (SPMD, identical on all cores via host-side column
rotation of x^T so each core's query rows always sit at columns 0:1024):
  - stream x^T in 4 blocks of 512 seq positions
  - K^T,V^T projections packed as one M=128 matmul chain (fp32r)
  - Q^T projection for the first 2 blocks (the core's query half)
  - V^T -> V via PE transpose (fp32)
  - scores^T[s,q] = K^T.T @ Q^T (contraction k=64), exp on ScalarE
    (scale 1/8 folded in), P^T kept fp32r
  - O^T[k,q] accumulated in PSUM over all 16 s-tiles with lhsT=[V|ones]
    (M=65; row 64 = softmax denominators)
  - normalize via reciprocal + one Newton step + K=1 broadcast matmul
"""
import sys
import types

import numpy as np

if "/opt/trn_rl_repo" not in sys.path:
    sys.path.insert(0, "/opt/trn_rl_repo")

if "antenv.axon_hooks" not in sys.modules:
    _hook = [None]
    _m = types.ModuleType("antenv.axon_hooks")
    _m.set_axon_ntff_profile_hook = lambda h: _hook.__setitem__(0, h)
    _m.get_axon_ntff_profile_hook = lambda: _hook[0]
    sys.modules["antenv.axon_hooks"] = _m

import concourse.bass as bass
import concourse.mybir as mybir
import concourse.tile as tile
from concourse import bacc
from concourse.bass_utils import run_bass_kernel_spmd
from concourse.masks import make_identity

B, S, D, KQ = 4, 2048, 1024, 64
N_CORES = 8
CORES_PER_B = N_CORES // B          # 2
SQ = S // CORES_PER_B               # 1024 query rows per core
SBLK = 512                          # seq streaming block
NBLK = S // SBLK                    # 4
NBLK_Q = SQ // SBLK                 # 2 blocks hold this core's queries
DCH = D // 128                      # 8 contraction chunks
NT = S // 128                       # 16 seq 128-tiles
QN = SQ // 512                      # 2 query N-tiles
SCALE = 1.0 / float(np.sqrt(KQ))

FP32R = mybir.dt.float32r
FP32 = mybir.dt.float32

TRACE = False                       # test harness sets True for NTFF timing
_CACHE = {}


def _build(use_bias: bool):
    nc = bacc.Bacc(trn_type="TRN2", target_bir_lowering=False, debug=False,
                   num_devices=N_CORES)
    xT = nc.dram_tensor("xT", [D, S], FP32R, kind="ExternalInput").ap()
    wkv = nc.dram_tensor("wkv", [D, 128], FP32R, kind="ExternalInput").ap()
    wq = nc.dram_tensor("wq", [D, KQ], FP32R, kind="ExternalInput").ap()
    onescol = nc.dram_tensor("onescol", [128, NT, 1], FP32R, kind="ExternalInput").ap()
    ones64 = nc.dram_tensor("ones64", [1, KQ], FP32R, kind="ExternalInput").ap()
    if use_bias:
        bkT = nc.dram_tensor("bkT", [KQ, S], FP32R, kind="ExternalInput").ap()
        bqT = nc.dram_tensor("bqT", [KQ, SQ], FP32R, kind="ExternalInput").ap()
        bvr = nc.dram_tensor("bvr", [128, NT, KQ], FP32R, kind="ExternalInput").ap()
    outT = nc.dram_tensor("outT", [KQ, SQ], FP32, kind="ExternalOutput").ap()

    xT_v = xT.rearrange("(c p) s -> p c s", p=128)
    wkv_v = wkv.rearrange("(c p) m -> p c m", p=128)
    wq_v = wq.rearrange("(c p) m -> p c m", p=128)

    with tile.TileContext(nc) as tc, \
         nc.allow_low_precision(reason="fp32r matmul operands are intentional"):
        with tc.tile_pool(name="xp", bufs=3) as xp, \
             tc.tile_pool(name="singles", bufs=1) as singles, \
             tc.tile_pool(name="vstg", bufs=2) as vstg, \
             tc.tile_pool(name="pp", bufs=3) as pp, \
             tc.tile_pool(name="fin", bufs=1) as fin, \
             tc.tile_pool(name="psA", bufs=1, space="PSUM") as psA, \
             tc.tile_pool(name="psVT", bufs=1, space="PSUM") as psVT, \
             tc.tile_pool(name="psS", bufs=2, space="PSUM") as psS, \
             tc.tile_pool(name="psO", bufs=1, space="PSUM") as psO:

            # ---- constants / persistent buffers ----
            wkv_s = singles.tile([128, DCH, 128], FP32R)
            nc.sync.dma_start(wkv_s[:], wkv_v[:])
            wq_s = singles.tile([128, DCH, KQ], FP32R)
            nc.sync.dma_start(wq_s[:], wq_v[:])
            ident = singles.tile([KQ, KQ], FP32)
            make_identity(nc, ident[:])
            ones64_s = singles.tile([1, KQ], FP32R)
            nc.sync.dma_start(ones64_s[:], ones64[:])

            kT = singles.tile([KQ, S], FP32R)       # K^T, built incrementally
            qT = singles.tile([KQ, SQ], FP32R)      # Q^T
            v_sbuf = singles.tile([128, NT, KQ + 1], FP32R)  # [V | ones]
            nc.sync.dma_start(v_sbuf[:, :, KQ:KQ + 1], onescol[:])

            if use_bias:
                bkT_s = singles.tile([KQ, S], FP32R)
                nc.sync.dma_start(bkT_s[:], bkT[:])
                bqT_s = singles.tile([KQ, SQ], FP32R)
                nc.sync.dma_start(bqT_s[:], bqT[:])
                bvr_s = singles.tile([128, NT, KQ], FP32R)
                nc.sync.dma_start(bvr_s[:], bvr[:])

            def proj_block(blk):
                sl = slice(blk * SBLK, (blk + 1) * SBLK)
                xt = xp.tile([128, DCH, SBLK], FP32R, tag="xt")
                nc.sync.dma_start(xt[:], xT_v[:, :, sl])
                # K^T | V^T packed projection, M=128
                pkv = psA.tile([128, SBLK], FP32, tag="proj")
                for c in range(DCH):
                    nc.tensor.matmul(pkv[:], wkv_s[:, c, :], xt[:, c, :],
                                     start=(c == 0), stop=(c == DCH - 1))
                if use_bias:
                    nc.vector.tensor_add(kT[:, sl], pkv[0:KQ, :], bkT_s[:, sl])
                else:
                    nc.vector.tensor_copy(kT[:, sl], pkv[0:KQ, :])
                vt_stage = vstg.tile([KQ, SBLK], FP32, tag="vt_stage")
                nc.vector.tensor_copy(vt_stage[:], pkv[KQ:128, :])
                # Q^T projection (only the first NBLK_Q blocks hold queries)
                if blk < NBLK_Q:
                    pq = psA.tile([128, SBLK], FP32, tag="proj")
                    for c in range(DCH):
                        nc.tensor.matmul(pq[0:KQ, :], wq_s[:, c, :], xt[:, c, :],
                                         start=(c == 0), stop=(c == DCH - 1))
                    if use_bias:
                        nc.vector.tensor_add(qT[:, sl], pq[0:KQ, :], bqT_s[:, sl])
                    else:
                        nc.vector.tensor_copy(qT[:, sl], pq[0:KQ, :])
                # V^T -> V (natural layout) via PE transpose
                for t in range(SBLK // 128):
                    st = blk * (SBLK // 128) + t
                    pvt = psVT.tile([128, KQ], FP32, tag="vt")
                    nc.tensor.transpose(pvt[:], vt_stage[:, t * 128:(t + 1) * 128],
                                        ident[:])
                    if use_bias:
                        nc.vector.tensor_add(v_sbuf[:, st, 0:KQ], pvt[:],
                                             bvr_s[:, st, :])
                    else:
                        nc.vector.tensor_copy(v_sbuf[:, st, 0:KQ], pvt[:])

            po = psO.tile([128, SQ], FP32, tag="out")    # rows 0:65 used

            def attn_tile(st, first, last):
                ps_ = psS.tile([128, SQ], FP32, tag="score")
                for qn in range(QN):
                    qsl = slice(qn * 512, (qn + 1) * 512)
                    nc.tensor.matmul(ps_[:, qsl], kT[:, st * 128:(st + 1) * 128],
                                     qT[:, qsl], start=True, stop=True)
                pt = pp.tile([128, SQ], FP32R, tag="pt")
                nc.scalar.activation(pt[:], ps_[:], mybir.ActivationFunctionType.Exp,
                                     scale=SCALE)
                for qn in range(QN):
                    qsl = slice(qn * 512, (qn + 1) * 512)
                    nc.tensor.matmul(po[0:KQ + 1, qsl], v_sbuf[:, st, :],
                                     pt[:, qsl], start=first, stop=last)

            # ---- emission order: proj b0,b1 -> att 0..7 -> proj b2 ->
            #      att 8..11 -> proj b3 -> att 12..15 ----
            proj_block(0)
            proj_block(1)
            for st in range(8):
                attn_tile(st, st == 0, False)
            proj_block(2)
            for st in range(8, 12):
                attn_tile(st, False, False)
            proj_block(3)
            for st in range(12, NT):
                attn_tile(st, False, st == NT - 1)

            # ---- normalize: out = O^T * (1/rowsum) broadcast over k ----
            r1 = fin.tile([1, SQ], FP32)
            nc.vector.reciprocal(r1[:], po[KQ:KQ + 1, :])
            t1 = fin.tile([1, SQ], FP32)
            nc.vector.tensor_mul(t1[:], r1[:], po[KQ:KQ + 1, :])
            u1 = fin.tile([1, SQ], FP32)
            nc.vector.tensor_scalar(u1[:], t1[:], -1.0, 2.0,
                                    mybir.AluOpType.mult, mybir.AluOpType.add)
            r2 = fin.tile([1, SQ], FP32R)
            nc.vector.tensor_mul(r2[:], r1[:], u1[:])
            pb = psS.tile([KQ, SQ], FP32, tag="score")
            for qn in range(QN):
                qsl = slice(qn * 512, (qn + 1) * 512)
                nc.tensor.matmul(pb[:, qsl], ones64_s[:], r2[:, qsl],
                                 start=True, stop=True)
            onum = fin.tile([KQ, SQ], FP32)
            nc.vector.tensor_copy(onum[:], po[0:KQ, :])
            osb = fin.tile([KQ, SQ], FP32)
            nc.vector.tensor_mul(osb[:], onum[:], pb[:])
            nc.sync.dma_start(outT[:], osb[:])

    nc.compile()
    return nc


def _get_program(use_bias: bool):
    key = bool(use_bias)
    if key not in _CACHE:
        _CACHE[key] = _build(key)
    return _CACHE[key]


def kernel(x, Wq, Bq, Wk, Bk, Wv, Bv):
    x = np.ascontiguousarray(np.asarray(x, dtype=np.float32))
    Wq = np.ascontiguousarray(np.asarray(Wq, dtype=np.float32))
    Wk = np.ascontiguousarray(np.asarray(Wk, dtype=np.float32))
    Wv = np.ascontiguousarray(np.asarray(Wv, dtype=np.float32))
    Bq = np.asarray(Bq, dtype=np.float32)
    Bk = np.asarray(Bk, dtype=np.float32)
    Bv = np.asarray(Bv, dtype=np.float32)
    use_bias = bool(Bq.any() or Bk.any() or Bv.any())

    nc = _get_program(use_bias)

    wkv_np = np.ascontiguousarray(np.concatenate([Wk, Wv], axis=1))
    ones_nt = np.ones((128, NT, 1), np.float32)
    ones_64 = np.ones((1, KQ), np.float32)

    in_maps = []
    for c in range(N_CORES):
        b, h = divmod(c, CORES_PER_B)
        xTb = x[b].T                                  # [D, S]
        roll = h * SQ
        if roll:
            xTc = np.ascontiguousarray(
                np.concatenate([xTb[:, roll:], xTb[:, :roll]], axis=1))
        else:
            xTc = np.ascontiguousarray(xTb)
        m = {"xT": xTc, "wkv": wkv_np, "wq": Wq,
             "onescol": ones_nt, "ones64": ones_64}
        if use_bias:
            bkTb = Bk.T
            bvb = Bv
            if roll:
                bkTc = np.ascontiguousarray(
                    np.concatenate([bkTb[:, roll:], bkTb[:, :roll]], axis=1))
                bvc = np.ascontiguousarray(
                    np.concatenate([bvb[roll:], bvb[:roll]], axis=0))
            else:
                bkTc = np.ascontiguousarray(bkTb)
                bvc = bvb
            m["bkT"] = bkTc
            m["bqT"] = np.ascontiguousarray(Bq.T[:, roll:roll + SQ])
            m["bvr"] = np.ascontiguousarray(
                bvc.reshape(NT, 128, KQ).transpose(1, 0, 2))
        in_maps.append(m)

    res = run_bass_kernel_spmd(nc, in_maps, list(range(N_CORES)),
                               trace=TRACE, trace_cores=[0] if TRACE else None)
    if TRACE:
        kernel.last_exec_time_ns = res.exec_time_ns
        kernel.last_results = res

    out = np.empty((B, S, KQ), np.float32)
    for c in range(N_CORES):
        b, h = divmod(c, CORES_PER_B)
        out[b, h * SQ:(h + 1) * SQ, :] = res.results[c]["outT"].T
    return out
